# revision 14
# baseline (speedup 1.0000x reference)
"""Trainium2 Bass kernel for the EvolutionaryFeatureExtractor problem.

Computes (pssm[512,20], conservation[512], mi_matrix[512,512]) from an MSA
[2048, 512] of int32 tokens (0..19 amino acids, 20 = gap) and a pseudocount
scale pc[1].

Strategy (8 NeuronCores, SPMD, no collectives):
  - MI pair work is sharded over i-positions: core c owns positions
    13c..13c+12 of the first 100 (core 7 carries 4 dummy positions that the
    host drops).  Each core computes J-rows = X_slice^T @ X for its slice,
    where X is the one-hot [2048, 2000] over the first 100 positions,
    via PE matmuls on a bf16 one-hot built on-chip with is_equal compares.
  - MI reduces to entropies:  mi = ((U - V - W)/tot + ln tot)/ln 2 with
      U = sum_ab J ln J, V = sum_b RS ln RS, W = sum_a CS ln CS,
      RS/CS the within-block marginals, tot the pair count.  RS rows are
    obtained for free by interleaving a non-gap-indicator column into the
    stationary operand (21 columns per position).
  - PSSM/conservation counts are sharded over the 512 columns (64 per core)
    and computed with a ones-row matmul over the one-hot.
Host side only slices inputs per core and concatenates the outputs.
"""

import numpy as np
from contextlib import ExitStack

import concourse.bass as bass
import concourse.bacc as bacc
import concourse.tile as tile
from concourse import mybir
from concourse.bass_utils import run_bass_kernel_spmd

# problem geometry (hardcoded per contest rules)
N_SEQS = 2048
SEQ_LEN = 512
NAA = 20
MPOS = 100          # MI over first 100 positions
NCORES = 8
POS_PER_CORE = 12   # i-rows per core: 8*12 = 96; positions 96-99 j-sharded
MJ_PER_CORE = 13    # j-cols per core for the 96-99 row tile (8*13 = 104 >= 100)
CNT_PER_CORE = 64   # 512/8
P = 128
KCH = N_SEQS // P   # 16 K-chunks
NW = NAA + 1        # 20 one-hot cols + 1 non-gap col per position
LN2 = float(np.log(2.0))
LN20 = float(np.log(20.0))
EPS = 1e-10

f32 = mybir.dt.float32
bf16 = mybir.dt.bfloat16
i32 = mybir.dt.int32
Alu = mybir.AluOpType
Act = mybir.ActivationFunctionType

# M-tiles: position-aligned groups of the 12 owned positions
MT = [(0, 6), (6, 6)]
NTAIL = 4           # globally shared tail positions 96..99


def _emit_kernel(nc, tc, ctx, tensors):
    (msa100, msa_mi, msa_mj, msa_cnt, s_all, dmask, dmask2, pc,
     pssm_o, cons_o, mi_o, mi2_o) = tensors

    consts = ctx.enter_context(tc.tile_pool(name="consts", bufs=1))
    xp = ctx.enter_context(tc.tile_pool(name="xp", bufs=1))
    post = ctx.enter_context(tc.tile_pool(name="post", bufs=1))
    small = ctx.enter_context(tc.tile_pool(name="small", bufs=2))
    jpsum = ctx.enter_context(tc.tile_pool(name="jpsum", bufs=1, space="PSUM"))
    cspsum = ctx.enter_context(tc.tile_pool(name="cspsum", bufs=1, space="PSUM"))
    ppsum = ctx.enter_context(tc.tile_pool(name="ppsum", bufs=1, space="PSUM"))
    dpool = ctx.enter_context(tc.tile_pool(name="dscratch", bufs=1, space="DRAM"))

    KG = 8               # k-chunks per pipeline half

    # ---------------- big input DMAs first ----------------
    msa_mi_i = consts.tile([P, KCH, POS_PER_CORE], i32)
    nc.sync.dma_start(out=msa_mi_i[:], in_=msa_mi[:, :].rearrange("(k p) i -> p k i", p=P))
    msa_mj_i = consts.tile([P, KCH, MJ_PER_CORE], i32)
    nc.scalar.dma_start(out=msa_mj_i[:], in_=msa_mj[:, :].rearrange("(k p) i -> p k i", p=P))
    msa100_i = consts.tile([P, KCH, MPOS], i32)
    msa100_r = msa100[:, :].rearrange("(k p) i -> p k i", p=P)
    nc.sync.dma_start(out=msa100_i[:, 0:8, :], in_=msa100_r[:, 0:8, :])
    nc.scalar.dma_start(out=msa100_i[:, 8:16, :], in_=msa100_r[:, 8:16, :])

    # ---------------- small constant loads ----------------
    s_sb = consts.tile([P, 20], f32)
    nc.scalar.dma_start(out=s_sb[:], in_=s_all[:, :])
    dm = []
    for t, (p0, npos) in enumerate(MT):
        d = consts.tile([npos, MPOS], f32, tag=f"dm{t}")
        nc.scalar.dma_start(out=d[:], in_=dmask[p0:p0 + npos, :])
        dm.append(d)
    dm2 = consts.tile([NTAIL, MJ_PER_CORE], f32)
    nc.scalar.dma_start(out=dm2[:], in_=dmask2[:, :])
    ones_sb = consts.tile([P, 1], bf16)
    nc.vector.memset(ones_sb[:], 1.0)
    eps_sb = consts.tile([P, 1], f32)
    nc.vector.memset(eps_sb[:], EPS)

    # pc-dependent pssm scalars (tiny, independent: hoisted to the front)
    pcb = small.tile([CNT_PER_CORE, 1], f32)
    nc.sync.dma_start(out=pcb[:], in_=pc[:, :].broadcast_to([CNT_PER_CORE, 1]))
    den = small.tile([CNT_PER_CORE, 1], f32)
    nc.vector.tensor_scalar(out=den[:], in0=pcb[:], scalar1=0.2, scalar2=2048.0,
                            op0=Alu.mult, op1=Alu.add)
    invd = small.tile([CNT_PER_CORE, 1], f32)
    nc.vector.reciprocal(out=invd[:], in_=den[:])
    sc = small.tile([CNT_PER_CORE, 1], f32)
    nc.vector.tensor_scalar(out=sc[:], in0=invd[:], scalar1=20.0, scalar2=None, op0=Alu.mult)
    pcntb = small.tile([CNT_PER_CORE, 1], f32)
    nc.vector.tensor_scalar(out=pcntb[:], in0=pcb[:], scalar1=0.01, scalar2=None, op0=Alu.mult)

    # ------------- stationary one-hots, i-major (contiguous stationary) ------
    # one TT is_equal against a broadcast iota builds all 20 one-hot cols;
    # runs on gpsimd to keep DVE free for the xmov builds
    iota_sb = consts.tile([P, NAA], bf16)
    for c in range(NAA):
        nc.gpsimd.memset(iota_sb[:, c:c + 1], float(c))
    msa_mi_bf = consts.tile([P, KCH, POS_PER_CORE], bf16)
    nc.gpsimd.tensor_copy(out=msa_mi_bf[:].rearrange("p k i -> p (k i)"),
                          in_=msa_mi_i[:].rearrange("p k i -> p (k i)"))
    xstat = xp.tile([P, KCH, POS_PER_CORE, NW], bf16)

    xmovj = xp.tile([P, NW, KCH, MJ_PER_CORE], bf16)
    xstat96 = xp.tile([P, KCH, NTAIL, NW], bf16)

    # ---------------- moving one-hot, pipelined in two halves ----------------
    msa100_bf = consts.tile([P, KCH, MPOS], bf16)
    xmov = xp.tile([P, NAA, KCH, MPOS], bf16)
    xng = xp.tile([P, KCH, MPOS], bf16)

    def build_half(g):
        k0 = g * KG
        ks = slice(k0, k0 + KG)
        nc.vector.tensor_tensor(
            xstat[:, ks, :, 0:NAA],
            msa_mi_bf[:, ks, :, None].to_broadcast([P, KG, POS_PER_CORE, NAA]),
            iota_sb[:, None, None, :].to_broadcast([P, KG, POS_PER_CORE, NAA]),
            Alu.is_equal)
        nc.vector.tensor_scalar(out=xstat[:, ks, :, NAA], in0=msa_mi_bf[:, ks, :],
                                scalar1=float(NAA), scalar2=None, op0=Alu.is_lt)
        fin = msa100_bf[:, ks, :].rearrange("p k j -> p (k j)")
        nc.vector.tensor_copy(out=fin,
                              in_=msa100_i[:, ks, :].rearrange("p k j -> p (k j)"))
        for a in range(NAA):
            nc.vector.tensor_scalar(out=xmov[:, a, ks, :].rearrange("p k j -> p (k j)"),
                                    in0=fin, scalar1=float(a), scalar2=None, op0=Alu.is_equal)
        nc.vector.tensor_scalar(out=xng[:, ks, :].rearrange("p k j -> p (k j)"),
                                in0=fin, scalar1=float(NAA), scalar2=None, op0=Alu.is_lt)

    # ---------------- J matmuls + MI post, per M-tile ----------------
    def emit_mt_matmuls(t, interleave_builds=False):
        p0, npos = MT[t]
        mr = npos * NW
        jps = jpsum.tile([126, 4, 512], f32, tag="jps")
        csps = cspsum.tile([126, 128], f32, tag="csps")
        for k in range(KCH):
            if interleave_builds and k % KG == 0:
                build_half(k // KG)
            lhsT = xstat[:, k, p0:p0 + npos, :]
            for n in range(4):
                nc.tensor.matmul(jps[0:mr, n, 0:500], lhsT=lhsT,
                                 rhs=xmov[:, :, k, 25 * n:25 * n + 25].rearrange(
                                     "p a j -> p j a"),
                                 start=(k == 0), stop=(k == KCH - 1))
            nc.tensor.matmul(csps[0:mr, 0:MPOS], lhsT=lhsT, rhs=xng[:, k, :],
                             start=(k == 0), stop=(k == KCH - 1))
        return jps, csps

    def emit_mt_post(t, jps, csps):
        p0, npos = MT[t]
        mr = npos * NW
        # copy PSUM -> SBUF (flat a-major 2000 cols) split over ACT/DVE;
        # frees the psum slot quickly for the next user
        jsb = post.tile([126, 2000], f32, tag="jsb")
        jview = jsb[0:mr].rearrange("p (n c) -> p n c", n=4)
        nc.scalar.copy(out=jview[:, 0:2, :], in_=jps[0:mr, 0:2, 0:500])
        nc.vector.tensor_copy(out=jview[:, 2:4, :], in_=jps[0:mr, 2:4, 0:500])
        eucg = post.tile([126, 3, MPOS], f32, tag="eucg")
        nc.scalar.copy(out=eucg[0:mr, 1, :], in_=csps[0:mr, 0:MPOS])
        # L = ln(J + eps);  E = J * L;  segmented sum over b
        lnj = post.tile([126, 2000], f32, tag="lnj")
        nc.scalar.activation(out=lnj[0:mr], in_=jsb[0:mr], func=Act.Ln,
                             bias=eps_sb[0:mr, 0:1], scale=1.0)
        ee = post.tile([126, 2000], f32, tag="ee")
        nc.vector.tensor_tensor(ee[0:mr], jsb[0:mr], lnj[0:mr], Alu.mult)
        nc.vector.tensor_reduce(out=eucg[0:mr, 0, :],
                                in_=ee[0:mr].rearrange("p (j b) -> p j b", b=NAA),
                                axis=mybir.AxisListType.X, op=Alu.add)
        lncs = post.tile([126, MPOS], f32, tag="lncs")
        nc.scalar.activation(out=lncs[0:mr], in_=eucg[0:mr, 1, :], func=Act.Ln,
                             bias=eps_sb[0:mr, 0:1], scale=1.0)
        nc.vector.tensor_tensor(eucg[0:mr, 2, :], eucg[0:mr, 1, :], lncs[0:mr], Alu.mult)

        # group sums via small matmuls:  psU rows=[U|-|W], psV rows=[V|tot|-]
        psU = ppsum.tile([6, 3, MPOS], f32, tag="psU")
        psV = ppsum.tile([6, 3, MPOS], f32, tag="psV")
        nc.tensor.matmul(psU[0:npos, :, :], lhsT=s_sb[0:mr, 0:6], rhs=eucg[0:mr, :, :],
                         start=True, stop=True)
        nc.tensor.matmul(psV[0:npos, :, :], lhsT=s_sb[0:mr, 6:12], rhs=eucg[0:mr, :, :],
                         start=True, stop=True)

        # mi = ((U - V - W) / max(tot,1) + ln tot) * dmask/ln2
        vt = small.tile([6, 2, MPOS], f32, tag="vt")
        nc.scalar.copy(out=vt[0:npos, :, :], in_=psV[0:npos, 0:2, :])
        tts = small.tile([6, MPOS], f32, tag="tts")
        nc.vector.tensor_scalar(out=tts[0:npos], in0=vt[0:npos, 1, :], scalar1=1.0,
                                scalar2=None, op0=Alu.max)
        inv = small.tile([6, MPOS], f32, tag="inv")
        nc.vector.reciprocal(out=inv[0:npos], in_=tts[0:npos])
        lnt = small.tile([6, MPOS], f32, tag="lnt")
        nc.scalar.activation(out=lnt[0:npos], in_=tts[0:npos], func=Act.Ln,
                             bias=0.0, scale=1.0)
        acc = small.tile([6, MPOS], f32, tag="acc")
        nc.vector.tensor_tensor(acc[0:npos], psU[0:npos, 0, :], vt[0:npos, 0, :], Alu.subtract)
        nc.vector.tensor_tensor(acc[0:npos], acc[0:npos], psU[0:npos, 2, :], Alu.subtract)
        nc.vector.tensor_tensor(acc[0:npos], acc[0:npos], inv[0:npos], Alu.mult)
        nc.vector.tensor_tensor(acc[0:npos], acc[0:npos], lnt[0:npos], Alu.add)
        nc.vector.tensor_tensor(acc[0:npos], acc[0:npos], dm[t][0:npos, :], Alu.mult)
        nc.sync.dma_start(out=mi_o[p0:p0 + npos, :], in_=acc[0:npos])

    # mt0 with builds interleaved into its k-loop
    jps0, csps0 = emit_mt_matmuls(0, interleave_builds=True)

    # counts one-hot build (DVE, overlaps mt0 PE work)
    msa_cnt_i = consts.tile([P, KCH, CNT_PER_CORE], i32)
    nc.scalar.dma_start(out=msa_cnt_i[:], in_=msa_cnt[:, :].rearrange("(k p) i -> p k i", p=P))
    msa_cnt_bf = consts.tile([P, KCH, CNT_PER_CORE], bf16)
    nc.gpsimd.tensor_copy(out=msa_cnt_bf[:].rearrange("p k i -> p (k i)"),
                          in_=msa_cnt_i[:].rearrange("p k i -> p (k i)"))
    xcnt = xp.tile([P, NAA, KCH, CNT_PER_CORE], bf16)
    cin = msa_cnt_bf[:].rearrange("p k j -> p (k j)")
    for a in range(NAA):
        nc.vector.tensor_scalar(out=xcnt[:, a, :, :].rearrange("p k j -> p (k j)"),
                                in0=cin, scalar1=float(a), scalar2=None, op0=Alu.is_equal)

    emit_mt_post(0, jps0, csps0)

    # ---------------- counts matmul (ones row), second on PE ----------
    cnt_ps = jpsum.tile([1, 3, 512], f32, tag="jps")
    CNT_NT = [(0, 8), (8, 8), (16, 4)]
    for k in range(KCH):
        for ni, (a0, aw) in enumerate(CNT_NT):
            nc.tensor.matmul(cnt_ps[0:1, ni, 0:aw * CNT_PER_CORE],
                             lhsT=ones_sb[:, 0:1],
                             rhs=xcnt[:, a0:a0 + aw, k, :],
                             start=(k == 0), stop=(k == KCH - 1))
    cnts = post.tile([1, 1280], f32)
    nc.scalar.copy(out=cnts[:],
                   in_=cnt_ps[0:1, :, :].rearrange("p a b -> p (a b)")[:, 0:1280])
    cnt_dram = dpool.tile([CNT_PER_CORE, NAA], f32)
    nc.scalar.dma_start(out=cnt_dram[:, :].rearrange("l a -> a l"),
                        in_=cnts[0:1, :].rearrange("p (a l) -> p a l", a=NAA))
    cnt64 = post.tile([CNT_PER_CORE, NAA], f32)
    nc.scalar.dma_start(out=cnt64[:], in_=cnt_dram[:, :])

    # tail-tile operand builds (DVE, overlap counts/mt1 PE work)
    msa_mj_bf = consts.tile([P, KCH, MJ_PER_CORE], bf16)
    nc.gpsimd.tensor_copy(out=msa_mj_bf[:].rearrange("p k i -> p (k i)"),
                          in_=msa_mj_i[:].rearrange("p k i -> p (k i)"))
    sjn = msa_mj_bf[:].rearrange("p k i -> p (k i)")
    for c in range(NAA):
        nc.vector.tensor_scalar(out=xmovj[:, c, :, :].rearrange("p k i -> p (k i)"),
                                in0=sjn, scalar1=float(c), scalar2=None, op0=Alu.is_equal)
    nc.vector.tensor_scalar(out=xmovj[:, NAA, :, :].rearrange("p k i -> p (k i)"),
                            in0=sjn, scalar1=float(NAA), scalar2=None, op0=Alu.is_lt)
    nc.vector.tensor_tensor(
        xstat96[:, :, :, 0:NAA],
        msa100_bf[:, :, 96:100, None].to_broadcast([P, KCH, NTAIL, NAA]),
        iota_sb[:, None, None, :].to_broadcast([P, KCH, NTAIL, NAA]),
        Alu.is_equal)
    nc.vector.tensor_scalar(out=xstat96[:, :, :, NAA], in0=msa100_bf[:, :, 96:100],
                            scalar1=float(NAA), scalar2=None, op0=Alu.is_lt)

    # mt1
    jps1, csps1 = emit_mt_matmuls(1)
    emit_mt_post(1, jps1, csps1)

    # ---------------- tail tile: rows 96..99, cols = this core's 13 j ------
    jps2 = cspsum.tile([NTAIL * NW, 288], f32, tag="csps")
    for k in range(KCH):
        nc.tensor.matmul(jps2[:, 0:NW * MJ_PER_CORE],
                         lhsT=xstat96[:, k, :, :],
                         rhs=xmovj[:, :, k, :],
                         start=(k == 0), stop=(k == KCH - 1))

    # post for the tail tile  (cols: c-major (c,j), rows: c-major (c,u))
    MJ = MJ_PER_CORE
    w2 = NW * MJ
    j2sb = post.tile([NTAIL * NW, w2], f32, tag="j2sb")
    nc.scalar.copy(out=j2sb[:, :], in_=jps2[:, 0:w2])
    lnj2 = post.tile([NTAIL * NW, w2], f32, tag="lnj2")
    nc.scalar.activation(out=lnj2[:], in_=j2sb[:], func=Act.Ln,
                         bias=eps_sb[0:NTAIL * NW, 0:1], scale=1.0)
    ee2 = post.tile([NTAIL * NW, w2], f32, tag="ee2")
    nc.vector.tensor_tensor(ee2[:], j2sb[:], lnj2[:], Alu.mult)
    eucg2 = post.tile([NTAIL * NW, 3, MJ], f32, tag="eucg2")
    nc.vector.tensor_reduce(out=eucg2[:, 0, :],
                            in_=ee2[:].rearrange("p (b j) -> p j b", b=NW)[:, :, 0:NAA],
                            axis=mybir.AxisListType.X, op=Alu.add)
    nc.vector.tensor_copy(out=eucg2[:, 1, :], in_=j2sb[:, NAA * MJ:NW * MJ])
    lncs2 = post.tile([NTAIL * NW, MJ], f32, tag="lncs2")
    nc.scalar.activation(out=lncs2[:], in_=eucg2[:, 1, :], func=Act.Ln,
                         bias=eps_sb[0:NTAIL * NW, 0:1], scale=1.0)
    nc.vector.tensor_tensor(eucg2[:, 2, :], eucg2[:, 1, :], lncs2[:], Alu.mult)
    psU2 = ppsum.tile([NTAIL, 3, MJ], f32, tag="psU")
    psV2 = ppsum.tile([NTAIL, 3, MJ], f32, tag="psV")
    nc.tensor.matmul(psU2[:, :, :], lhsT=s_sb[0:NTAIL * NW, 12:16], rhs=eucg2[:, :, :],
                     start=True, stop=True)
    nc.tensor.matmul(psV2[:, :, :], lhsT=s_sb[0:NTAIL * NW, 16:20], rhs=eucg2[:, :, :],
                     start=True, stop=True)
    vt2 = small.tile([NTAIL, 2, MJ], f32, tag="vt2")
    nc.scalar.copy(out=vt2[:, :, :], in_=psV2[:, 0:2, :])
    tts2 = small.tile([NTAIL, MJ], f32, tag="tts2")
    nc.vector.tensor_scalar(out=tts2[:], in0=vt2[:, 1, :], scalar1=1.0,
                            scalar2=None, op0=Alu.max)
    inv2 = small.tile([NTAIL, MJ], f32, tag="inv2")
    nc.vector.reciprocal(out=inv2[:], in_=tts2[:])
    lnt2 = small.tile([NTAIL, MJ], f32, tag="lnt2")
    nc.scalar.activation(out=lnt2[:], in_=tts2[:], func=Act.Ln, bias=0.0, scale=1.0)
    acc2 = small.tile([NTAIL, MJ], f32, tag="acc2")
    nc.vector.tensor_tensor(acc2[:], psU2[:, 0, :], vt2[:, 0, :], Alu.subtract)
    nc.vector.tensor_tensor(acc2[:], acc2[:], psU2[:, 2, :], Alu.subtract)
    nc.vector.tensor_tensor(acc2[:], acc2[:], inv2[:], Alu.mult)
    nc.vector.tensor_tensor(acc2[:], acc2[:], lnt2[:], Alu.add)
    nc.vector.tensor_tensor(acc2[:], acc2[:], dm2[:, :], Alu.mult)
    nc.sync.dma_start(out=mi2_o[:, :], in_=acc2[:])

    # ---------------- pssm (overlaps mt1/tail PE work) ----------------
    cntp = small.tile([CNT_PER_CORE, NAA], f32)
    nc.vector.tensor_scalar(out=cntp[:], in0=cnt64[:], scalar1=pcntb[:, 0:1],
                            scalar2=None, op0=Alu.add)
    pssm_sb = small.tile([CNT_PER_CORE, NAA], f32)
    nc.scalar.activation(out=pssm_sb[:], in_=cntp[:], func=Act.Ln,
                         bias=eps_sb[0:CNT_PER_CORE, 0:1], scale=sc[:, 0:1])
    nc.sync.dma_start(out=pssm_o[:, :], in_=pssm_sb[:])

    # ---------------- conservation ----------------
    total = small.tile([CNT_PER_CORE, 1], f32)
    nc.vector.tensor_reduce(out=total[:], in_=cnt64[:], axis=mybir.AxisListType.X, op=Alu.add)
    tots = small.tile([CNT_PER_CORE, 1], f32)
    nc.vector.tensor_scalar(out=tots[:], in0=total[:], scalar1=1.0, scalar2=None, op0=Alu.max)
    invt = small.tile([CNT_PER_CORE, 1], f32)
    nc.vector.reciprocal(out=invt[:], in_=tots[:])
    ffreq = small.tile([CNT_PER_CORE, NAA], f32)
    nc.vector.tensor_scalar(out=ffreq[:], in0=cnt64[:], scalar1=invt[:, 0:1],
                            scalar2=None, op0=Alu.mult)
    lf = small.tile([CNT_PER_CORE, NAA], f32)
    nc.scalar.activation(out=lf[:], in_=ffreq[:], func=Act.Ln,
                         bias=eps_sb[0:CNT_PER_CORE, 0:1], scale=1.0)
    fl = small.tile([CNT_PER_CORE, NAA], f32)
    nc.vector.tensor_tensor(fl[:], ffreq[:], lf[:], Alu.mult)
    se = small.tile([CNT_PER_CORE, 1], f32)
    nc.vector.tensor_reduce(out=se[:], in_=fl[:], axis=mybir.AxisListType.X, op=Alu.add)
    consv = small.tile([CNT_PER_CORE, 1], f32)
    # cons = 1 + (sum f ln f)/ln(20)
    nc.vector.tensor_scalar(out=consv[:], in0=se[:], scalar1=1.0 / LN20, scalar2=1.0,
                            op0=Alu.mult, op1=Alu.add)
    mask = small.tile([CNT_PER_CORE, 1], f32)
    nc.vector.tensor_scalar(out=mask[:], in0=total[:], scalar1=0.0, scalar2=None, op0=Alu.is_gt)
    nc.vector.tensor_tensor(consv[:], consv[:], mask[:], Alu.mult)
    nc.sync.dma_start(out=cons_o[:, :], in_=consv[:])


_NC_CACHE = None


def _build_nc():
    global _NC_CACHE
    if _NC_CACHE is not None:
        return _NC_CACHE
    nc = bacc.Bacc("TRN2", target_bir_lowering=False)
    msa100 = nc.dram_tensor("msa100", [N_SEQS, MPOS], i32, kind="ExternalInput")
    msa_mi = nc.dram_tensor("msa_mi", [N_SEQS, POS_PER_CORE], i32, kind="ExternalInput")
    msa_mj = nc.dram_tensor("msa_mj", [N_SEQS, MJ_PER_CORE], i32, kind="ExternalInput")
    msa_cnt = nc.dram_tensor("msa_cnt", [N_SEQS, CNT_PER_CORE], i32, kind="ExternalInput")
    s_all = nc.dram_tensor("s_all", [P, 20], f32, kind="ExternalInput")
    dmask = nc.dram_tensor("dmask", [POS_PER_CORE, MPOS], f32, kind="ExternalInput")
    dmask2 = nc.dram_tensor("dmask2", [NTAIL, MJ_PER_CORE], f32, kind="ExternalInput")
    pc = nc.dram_tensor("pc", [1, 1], f32, kind="ExternalInput")
    pssm_o = nc.dram_tensor("pssm_part", [CNT_PER_CORE, NAA], f32, kind="ExternalOutput")
    cons_o = nc.dram_tensor("cons_part", [CNT_PER_CORE, 1], f32, kind="ExternalOutput")
    mi_o = nc.dram_tensor("mi_part", [POS_PER_CORE, MPOS], f32, kind="ExternalOutput")
    mi2_o = nc.dram_tensor("mi2_part", [NTAIL, MJ_PER_CORE], f32, kind="ExternalOutput")
    with tile.TileContext(nc) as tc:
        with ExitStack() as ctx:
            _emit_kernel(nc, tc, ctx,
                         (msa100, msa_mi, msa_mj, msa_cnt, s_all, dmask, dmask2, pc,
                          pssm_o, cons_o, mi_o, mi2_o))
    nc.compile()
    _NC_CACHE = nc
    return nc


def _host_inputs(msa, pc):
    msa = np.ascontiguousarray(np.asarray(msa), dtype=np.int32)
    pc_np = np.asarray(pc, dtype=np.float32).reshape(1, 1)
    # i-major S matrix: psum row r = m*21 + c
    s_arr = np.zeros((P, 20), np.float32)
    for m in range(6):                       # npos=6 tiles
        s_arr[NW * m: NW * m + NAA, m] = 1.0
        s_arr[NW * m + NAA, 6 + m] = 1.0
    for m in range(NTAIL):                   # npos=4 tail tile
        s_arr[NW * m: NW * m + NAA, 12 + m] = 1.0
        s_arr[NW * m + NAA, 16 + m] = 1.0
    msa100 = np.ascontiguousarray(msa[:, :MPOS])
    in_maps = []
    for c in range(NCORES):
        jcols = [(MJ_PER_CORE * c + t) if (MJ_PER_CORE * c + t) < MPOS else 0
                 for t in range(MJ_PER_CORE)]
        dmask = np.full((POS_PER_CORE, MPOS), 1.0 / LN2, np.float32)
        for t in range(POS_PER_CORE):
            dmask[t, POS_PER_CORE * c + t] = 0.0
        dmask2 = np.full((NTAIL, MJ_PER_CORE), 1.0 / LN2, np.float32)
        for u in range(NTAIL):
            for t in range(MJ_PER_CORE):
                if jcols[t] == 96 + u and MJ_PER_CORE * c + t < MPOS:
                    dmask2[u, t] = 0.0
        in_maps.append({
            "msa100": msa100,
            "msa_mi": np.ascontiguousarray(
                msa[:, POS_PER_CORE * c: POS_PER_CORE * (c + 1)]),
            "msa_mj": np.ascontiguousarray(msa[:, jcols]),
            "msa_cnt": np.ascontiguousarray(msa[:, CNT_PER_CORE * c: CNT_PER_CORE * (c + 1)]),
            "s_all": s_arr,
            "dmask": dmask,
            "dmask2": dmask2,
            "pc": pc_np,
        })
    return in_maps


def _run(msa, pc, **spmd_kwargs):
    nc = _build_nc()
    in_maps = _host_inputs(msa, pc)
    res = run_bass_kernel_spmd(nc, in_maps, core_ids=list(range(NCORES)), **spmd_kwargs)
    pssm = np.concatenate([res.results[c]["pssm_part"] for c in range(NCORES)], axis=0)
    cons = np.concatenate([res.results[c]["cons_part"][:, 0] for c in range(NCORES)], axis=0)
    rows = np.concatenate([res.results[c]["mi_part"] for c in range(NCORES)], axis=0)
    mi = np.zeros((SEQ_LEN, SEQ_LEN), np.float32)
    mi[:NCORES * POS_PER_CORE, :MPOS] = rows
    for c in range(NCORES):
        m2 = res.results[c]["mi2_part"]          # [4, 13]
        for t in range(MJ_PER_CORE):
            jc = MJ_PER_CORE * c + t
            if jc < MPOS:
                mi[96:MPOS, jc] = m2[:, t]
    return (pssm.astype(np.float32), cons.astype(np.float32), mi), res


def kernel(msa, pc):
    out, _ = _run(msa, pc)
    return out


# revision 15
# speedup vs baseline: 1.8067x; 1.8067x over previous
"""Trainium2 Bass kernel for the EvolutionaryFeatureExtractor problem.

Computes (pssm[512,20], conservation[512], mi_matrix[512,512]) from an MSA
[2048, 512] of int32 tokens (0..19 amino acids, 20 = gap) and a pseudocount
scale pc[1].

Strategy (8 NeuronCores, SPMD, no collectives):
  - MI pair work is sharded over i-positions: core c owns positions
    13c..13c+12 of the first 100 (core 7 carries 4 dummy positions that the
    host drops).  Each core computes J-rows = X_slice^T @ X for its slice,
    where X is the one-hot [2048, 2000] over the first 100 positions,
    via PE matmuls on a bf16 one-hot built on-chip with is_equal compares.
  - MI reduces to entropies:  mi = ((U - V - W)/tot + ln tot)/ln 2 with
      U = sum_ab J ln J, V = sum_b RS ln RS, W = sum_a CS ln CS,
      RS/CS the within-block marginals, tot the pair count.  RS rows are
    obtained for free by interleaving a non-gap-indicator column into the
    stationary operand (21 columns per position).
  - PSSM/conservation counts are sharded over the 512 columns (64 per core)
    and computed with a ones-row matmul over the one-hot.
Host side only slices inputs per core and concatenates the outputs.
"""

import numpy as np
from contextlib import ExitStack

import concourse.bass as bass
import concourse.bacc as bacc
import concourse.tile as tile
from concourse import mybir
from concourse.bass_utils import run_bass_kernel_spmd

# problem geometry (hardcoded per contest rules)
N_SEQS = 2048
SEQ_LEN = 512
NAA = 20
MPOS = 100          # MI over first 100 positions
NCORES = 8
POS_PER_CORE = 12   # i-rows per core: 8*12 = 96; positions 96-99 j-sharded
MJ_PER_CORE = 13    # j-cols per core for the 96-99 row tile (8*13 = 104 >= 100)
CNT_PER_CORE = 64   # 512/8
P = 128
KCH = N_SEQS // P   # 16 K-chunks
NW = NAA + 1        # 20 one-hot cols + 1 non-gap col per position
LN2 = float(np.log(2.0))
LN20 = float(np.log(20.0))
EPS = 1e-10

f32 = mybir.dt.float32
bf16 = mybir.dt.bfloat16
i32 = mybir.dt.int32
Alu = mybir.AluOpType
Act = mybir.ActivationFunctionType

# M-tiles: position-aligned groups of the 12 owned positions
MT = [(0, 6), (6, 6)]
NTAIL = 4           # globally shared tail positions 96..99


def _emit_kernel(nc, tc, ctx, tensors):
    (msa100, msa_mi, msa_mj, msa_cnt, s_all, dmask, dmask2, pc,
     pssm_o, cons_o, mi_o, mi2_o) = tensors

    consts = ctx.enter_context(tc.tile_pool(name="consts", bufs=1))
    xp = ctx.enter_context(tc.tile_pool(name="xp", bufs=1))
    post = ctx.enter_context(tc.tile_pool(name="post", bufs=1))
    small = ctx.enter_context(tc.tile_pool(name="small", bufs=2))
    jpsum = ctx.enter_context(tc.tile_pool(name="jpsum", bufs=1, space="PSUM"))
    cspsum = ctx.enter_context(tc.tile_pool(name="cspsum", bufs=1, space="PSUM"))
    ppsum = ctx.enter_context(tc.tile_pool(name="ppsum", bufs=1, space="PSUM"))
    dpool = ctx.enter_context(tc.tile_pool(name="dscratch", bufs=1, space="DRAM"))

    KG = 8               # k-chunks per pipeline half

    # ---------------- big input DMAs first ----------------
    msa_mi_i = consts.tile([P, KCH, POS_PER_CORE], i32)
    nc.sync.dma_start(out=msa_mi_i[:], in_=msa_mi[:, :].rearrange("(k p) i -> p k i", p=P))
    msa_mj_i = consts.tile([P, KCH, MJ_PER_CORE], i32)
    nc.scalar.dma_start(out=msa_mj_i[:], in_=msa_mj[:, :].rearrange("(k p) i -> p k i", p=P))
    msa100_i = consts.tile([P, KCH, MPOS], i32)
    msa100_r = msa100[:, :].rearrange("(k p) i -> p k i", p=P)
    nc.sync.dma_start(out=msa100_i[:, 0:8, :], in_=msa100_r[:, 0:8, :])
    nc.scalar.dma_start(out=msa100_i[:, 8:16, :], in_=msa100_r[:, 8:16, :])

    # ---------------- small constant loads ----------------
    s_sb = consts.tile([P, 20], f32)
    nc.scalar.dma_start(out=s_sb[:], in_=s_all[:, :])
    dm = []
    for t, (p0, npos) in enumerate(MT):
        d = consts.tile([npos, MPOS], f32, tag=f"dm{t}")
        nc.scalar.dma_start(out=d[:], in_=dmask[p0:p0 + npos, :])
        dm.append(d)
    dm2 = consts.tile([NTAIL, MJ_PER_CORE], f32)
    nc.scalar.dma_start(out=dm2[:], in_=dmask2[:, :])
    ones_sb = consts.tile([P, 1], bf16)
    nc.vector.memset(ones_sb[:], 1.0)
    eps_sb = consts.tile([P, 1], f32)
    nc.vector.memset(eps_sb[:], EPS)

    # pc-dependent pssm scalars (tiny, independent: hoisted to the front)
    pcb = small.tile([CNT_PER_CORE, 1], f32)
    nc.sync.dma_start(out=pcb[:], in_=pc[:, :].broadcast_to([CNT_PER_CORE, 1]))
    den = small.tile([CNT_PER_CORE, 1], f32)
    nc.vector.tensor_scalar(out=den[:], in0=pcb[:], scalar1=0.2, scalar2=2048.0,
                            op0=Alu.mult, op1=Alu.add)
    invd = small.tile([CNT_PER_CORE, 1], f32)
    nc.vector.reciprocal(out=invd[:], in_=den[:])
    sc = small.tile([CNT_PER_CORE, 1], f32)
    nc.vector.tensor_scalar(out=sc[:], in0=invd[:], scalar1=20.0, scalar2=None, op0=Alu.mult)
    pcntb = small.tile([CNT_PER_CORE, 1], f32)
    nc.vector.tensor_scalar(out=pcntb[:], in0=pcb[:], scalar1=0.01, scalar2=None, op0=Alu.mult)

    # ------------- stationary one-hots, i-major (contiguous stationary) ------
    # one TT is_equal against a broadcast iota builds all 20 one-hot cols;
    # runs on gpsimd to keep DVE free for the xmov builds
    iota_sb = consts.tile([P, NAA], bf16)
    for c in range(NAA):
        nc.gpsimd.memset(iota_sb[:, c:c + 1], float(c))
    msa_mi_bf = consts.tile([P, KCH, POS_PER_CORE], bf16)
    nc.gpsimd.tensor_copy(out=msa_mi_bf[:].rearrange("p k i -> p (k i)"),
                          in_=msa_mi_i[:].rearrange("p k i -> p (k i)"))
    xstat = xp.tile([P, KCH, POS_PER_CORE, NW], bf16)

    xmovj = xp.tile([P, NW, KCH, MJ_PER_CORE], bf16)
    xstat96 = xp.tile([P, KCH, NTAIL, NW], bf16)

    # ---------------- moving one-hot, pipelined in two halves ----------------
    msa100_bf = consts.tile([P, KCH, MPOS], bf16)
    xmov = xp.tile([P, NAA, KCH, MPOS], bf16)
    xng = xp.tile([P, KCH, MPOS], bf16)

    def build_half(g):
        k0 = g * KG
        ks = slice(k0, k0 + KG)
        nc.vector.tensor_tensor(
            xstat[:, ks, :, 0:NAA],
            msa_mi_bf[:, ks, :, None].to_broadcast([P, KG, POS_PER_CORE, NAA]),
            iota_sb[:, None, None, :].to_broadcast([P, KG, POS_PER_CORE, NAA]),
            Alu.is_equal)
        nc.vector.tensor_scalar(out=xstat[:, ks, :, NAA], in0=msa_mi_bf[:, ks, :],
                                scalar1=float(NAA), scalar2=None, op0=Alu.is_lt)
        fin = msa100_bf[:, ks, :].rearrange("p k j -> p (k j)")
        nc.vector.tensor_copy(out=fin,
                              in_=msa100_i[:, ks, :].rearrange("p k j -> p (k j)"))
        for a in range(NAA):
            nc.vector.tensor_scalar(out=xmov[:, a, ks, :].rearrange("p k j -> p (k j)"),
                                    in0=fin, scalar1=float(a), scalar2=None, op0=Alu.is_equal)
        nc.vector.tensor_scalar(out=xng[:, ks, :].rearrange("p k j -> p (k j)"),
                                in0=fin, scalar1=float(NAA), scalar2=None, op0=Alu.is_lt)

    # ---------------- J matmuls + MI post, per M-tile ----------------
    def emit_mt_matmuls(t, interleave_builds=False):
        p0, npos = MT[t]
        mr = npos * NW
        jps = jpsum.tile([126, 4, 512], f32, tag="jps")
        csps = cspsum.tile([126, 128], f32, tag="csps")
        for k in range(KCH):
            if interleave_builds and k % KG == 0:
                build_half(k // KG)
            lhsT = xstat[:, k, p0:p0 + npos, :]
            for n in range(4):
                nc.tensor.matmul(jps[0:mr, n, 0:500], lhsT=lhsT,
                                 rhs=xmov[:, 5 * n:5 * n + 5, k, :],
                                 start=(k == 0), stop=(k == KCH - 1))
            nc.tensor.matmul(csps[0:mr, 0:MPOS], lhsT=lhsT, rhs=xng[:, k, :],
                             start=(k == 0), stop=(k == KCH - 1))
        return jps, csps

    def emit_mt_post(t, jps, csps):
        p0, npos = MT[t]
        mr = npos * NW
        # copy PSUM -> SBUF (flat a-major 2000 cols) split over ACT/DVE;
        # frees the psum slot quickly for the next user
        jsb = post.tile([126, 2000], f32, tag="jsb")
        jview = jsb[0:mr].rearrange("p (n c) -> p n c", n=4)
        nc.scalar.copy(out=jview[:, 0:2, :], in_=jps[0:mr, 0:2, 0:500])
        nc.vector.tensor_copy(out=jview[:, 2:4, :], in_=jps[0:mr, 2:4, 0:500])
        eucg = post.tile([126, 3, MPOS], f32, tag="eucg")
        nc.scalar.copy(out=eucg[0:mr, 1, :], in_=csps[0:mr, 0:MPOS])
        # L = ln(J + eps);  E = J * L;  segmented sum over b
        lnj = post.tile([126, 2000], f32, tag="lnj")
        nc.scalar.activation(out=lnj[0:mr], in_=jsb[0:mr], func=Act.Ln,
                             bias=eps_sb[0:mr, 0:1], scale=1.0)
        ee = post.tile([126, 2000], f32, tag="ee")
        nc.vector.tensor_tensor(ee[0:mr], jsb[0:mr], lnj[0:mr], Alu.mult)
        nc.vector.tensor_reduce(out=eucg[0:mr, 0, :],
                                in_=ee[0:mr].rearrange("p (b j) -> p j b", b=NAA),
                                axis=mybir.AxisListType.X, op=Alu.add)
        lncs = post.tile([126, MPOS], f32, tag="lncs")
        nc.scalar.activation(out=lncs[0:mr], in_=eucg[0:mr, 1, :], func=Act.Ln,
                             bias=eps_sb[0:mr, 0:1], scale=1.0)
        nc.vector.tensor_tensor(eucg[0:mr, 2, :], eucg[0:mr, 1, :], lncs[0:mr], Alu.mult)

        # group sums via small matmuls:  psU rows=[U|-|W], psV rows=[V|tot|-]
        psU = ppsum.tile([6, 3, MPOS], f32, tag="psU")
        psV = ppsum.tile([6, 3, MPOS], f32, tag="psV")
        nc.tensor.matmul(psU[0:npos, :, :], lhsT=s_sb[0:mr, 0:6], rhs=eucg[0:mr, :, :],
                         start=True, stop=True)
        nc.tensor.matmul(psV[0:npos, :, :], lhsT=s_sb[0:mr, 6:12], rhs=eucg[0:mr, :, :],
                         start=True, stop=True)

        # mi = ((U - V - W) / max(tot,1) + ln tot) * dmask/ln2
        vt = small.tile([6, 2, MPOS], f32, tag="vt")
        nc.scalar.copy(out=vt[0:npos, :, :], in_=psV[0:npos, 0:2, :])
        tts = small.tile([6, MPOS], f32, tag="tts")
        nc.vector.tensor_scalar(out=tts[0:npos], in0=vt[0:npos, 1, :], scalar1=1.0,
                                scalar2=None, op0=Alu.max)
        inv = small.tile([6, MPOS], f32, tag="inv")
        nc.vector.reciprocal(out=inv[0:npos], in_=tts[0:npos])
        lnt = small.tile([6, MPOS], f32, tag="lnt")
        nc.scalar.activation(out=lnt[0:npos], in_=tts[0:npos], func=Act.Ln,
                             bias=0.0, scale=1.0)
        acc = small.tile([6, MPOS], f32, tag="acc")
        nc.vector.tensor_tensor(acc[0:npos], psU[0:npos, 0, :], vt[0:npos, 0, :], Alu.subtract)
        nc.vector.tensor_tensor(acc[0:npos], acc[0:npos], psU[0:npos, 2, :], Alu.subtract)
        nc.vector.tensor_tensor(acc[0:npos], acc[0:npos], inv[0:npos], Alu.mult)
        nc.vector.tensor_tensor(acc[0:npos], acc[0:npos], lnt[0:npos], Alu.add)
        nc.vector.tensor_tensor(acc[0:npos], acc[0:npos], dm[t][0:npos, :], Alu.mult)
        nc.sync.dma_start(out=mi_o[p0:p0 + npos, :], in_=acc[0:npos])

    # mt0 with builds interleaved into its k-loop
    jps0, csps0 = emit_mt_matmuls(0, interleave_builds=True)

    # counts one-hot build (DVE, overlaps mt0 PE work)
    msa_cnt_i = consts.tile([P, KCH, CNT_PER_CORE], i32)
    nc.scalar.dma_start(out=msa_cnt_i[:], in_=msa_cnt[:, :].rearrange("(k p) i -> p k i", p=P))
    msa_cnt_bf = consts.tile([P, KCH, CNT_PER_CORE], bf16)
    nc.gpsimd.tensor_copy(out=msa_cnt_bf[:].rearrange("p k i -> p (k i)"),
                          in_=msa_cnt_i[:].rearrange("p k i -> p (k i)"))
    xcnt = xp.tile([P, NAA, KCH, CNT_PER_CORE], bf16)
    cin = msa_cnt_bf[:].rearrange("p k j -> p (k j)")
    for a in range(NAA):
        nc.vector.tensor_scalar(out=xcnt[:, a, :, :].rearrange("p k j -> p (k j)"),
                                in0=cin, scalar1=float(a), scalar2=None, op0=Alu.is_equal)

    emit_mt_post(0, jps0, csps0)

    # ---------------- counts matmul (ones row), second on PE ----------
    cnt_ps = jpsum.tile([1, 3, 512], f32, tag="jps")
    CNT_NT = [(0, 8), (8, 8), (16, 4)]
    for k in range(KCH):
        for ni, (a0, aw) in enumerate(CNT_NT):
            nc.tensor.matmul(cnt_ps[0:1, ni, 0:aw * CNT_PER_CORE],
                             lhsT=ones_sb[:, 0:1],
                             rhs=xcnt[:, a0:a0 + aw, k, :],
                             start=(k == 0), stop=(k == KCH - 1))
    cnts = post.tile([1, 1280], f32)
    nc.scalar.copy(out=cnts[:],
                   in_=cnt_ps[0:1, :, :].rearrange("p a b -> p (a b)")[:, 0:1280])
    cnt_dram = dpool.tile([CNT_PER_CORE, NAA], f32)
    nc.scalar.dma_start(out=cnt_dram[:, :].rearrange("l a -> a l"),
                        in_=cnts[0:1, :].rearrange("p (a l) -> p a l", a=NAA))
    cnt64 = post.tile([CNT_PER_CORE, NAA], f32)
    nc.scalar.dma_start(out=cnt64[:], in_=cnt_dram[:, :])

    # tail-tile operand builds (DVE, overlap counts/mt1 PE work)
    msa_mj_bf = consts.tile([P, KCH, MJ_PER_CORE], bf16)
    nc.gpsimd.tensor_copy(out=msa_mj_bf[:].rearrange("p k i -> p (k i)"),
                          in_=msa_mj_i[:].rearrange("p k i -> p (k i)"))
    sjn = msa_mj_bf[:].rearrange("p k i -> p (k i)")
    for c in range(NAA):
        nc.vector.tensor_scalar(out=xmovj[:, c, :, :].rearrange("p k i -> p (k i)"),
                                in0=sjn, scalar1=float(c), scalar2=None, op0=Alu.is_equal)
    nc.vector.tensor_scalar(out=xmovj[:, NAA, :, :].rearrange("p k i -> p (k i)"),
                            in0=sjn, scalar1=float(NAA), scalar2=None, op0=Alu.is_lt)
    nc.vector.tensor_tensor(
        xstat96[:, :, :, 0:NAA],
        msa100_bf[:, :, 96:100, None].to_broadcast([P, KCH, NTAIL, NAA]),
        iota_sb[:, None, None, :].to_broadcast([P, KCH, NTAIL, NAA]),
        Alu.is_equal)
    nc.vector.tensor_scalar(out=xstat96[:, :, :, NAA], in0=msa100_bf[:, :, 96:100],
                            scalar1=float(NAA), scalar2=None, op0=Alu.is_lt)

    # mt1
    jps1, csps1 = emit_mt_matmuls(1)
    emit_mt_post(1, jps1, csps1)

    # ---------------- tail tile: rows 96..99, cols = this core's 13 j ------
    jps2 = cspsum.tile([NTAIL * NW, 288], f32, tag="csps")
    for k in range(KCH):
        nc.tensor.matmul(jps2[:, 0:NW * MJ_PER_CORE],
                         lhsT=xstat96[:, k, :, :],
                         rhs=xmovj[:, :, k, :],
                         start=(k == 0), stop=(k == KCH - 1))

    # post for the tail tile  (cols: c-major (c,j), rows: c-major (c,u))
    MJ = MJ_PER_CORE
    w2 = NW * MJ
    j2sb = post.tile([NTAIL * NW, w2], f32, tag="j2sb")
    nc.scalar.copy(out=j2sb[:, :], in_=jps2[:, 0:w2])
    lnj2 = post.tile([NTAIL * NW, w2], f32, tag="lnj2")
    nc.scalar.activation(out=lnj2[:], in_=j2sb[:], func=Act.Ln,
                         bias=eps_sb[0:NTAIL * NW, 0:1], scale=1.0)
    ee2 = post.tile([NTAIL * NW, w2], f32, tag="ee2")
    nc.vector.tensor_tensor(ee2[:], j2sb[:], lnj2[:], Alu.mult)
    eucg2 = post.tile([NTAIL * NW, 3, MJ], f32, tag="eucg2")
    nc.vector.tensor_reduce(out=eucg2[:, 0, :],
                            in_=ee2[:].rearrange("p (b j) -> p j b", b=NW)[:, :, 0:NAA],
                            axis=mybir.AxisListType.X, op=Alu.add)
    nc.vector.tensor_copy(out=eucg2[:, 1, :], in_=j2sb[:, NAA * MJ:NW * MJ])
    lncs2 = post.tile([NTAIL * NW, MJ], f32, tag="lncs2")
    nc.scalar.activation(out=lncs2[:], in_=eucg2[:, 1, :], func=Act.Ln,
                         bias=eps_sb[0:NTAIL * NW, 0:1], scale=1.0)
    nc.vector.tensor_tensor(eucg2[:, 2, :], eucg2[:, 1, :], lncs2[:], Alu.mult)
    psU2 = ppsum.tile([NTAIL, 3, MJ], f32, tag="psU")
    psV2 = ppsum.tile([NTAIL, 3, MJ], f32, tag="psV")
    nc.tensor.matmul(psU2[:, :, :], lhsT=s_sb[0:NTAIL * NW, 12:16], rhs=eucg2[:, :, :],
                     start=True, stop=True)
    nc.tensor.matmul(psV2[:, :, :], lhsT=s_sb[0:NTAIL * NW, 16:20], rhs=eucg2[:, :, :],
                     start=True, stop=True)
    vt2 = small.tile([NTAIL, 2, MJ], f32, tag="vt2")
    nc.scalar.copy(out=vt2[:, :, :], in_=psV2[:, 0:2, :])
    tts2 = small.tile([NTAIL, MJ], f32, tag="tts2")
    nc.vector.tensor_scalar(out=tts2[:], in0=vt2[:, 1, :], scalar1=1.0,
                            scalar2=None, op0=Alu.max)
    inv2 = small.tile([NTAIL, MJ], f32, tag="inv2")
    nc.vector.reciprocal(out=inv2[:], in_=tts2[:])
    lnt2 = small.tile([NTAIL, MJ], f32, tag="lnt2")
    nc.scalar.activation(out=lnt2[:], in_=tts2[:], func=Act.Ln, bias=0.0, scale=1.0)
    acc2 = small.tile([NTAIL, MJ], f32, tag="acc2")
    nc.vector.tensor_tensor(acc2[:], psU2[:, 0, :], vt2[:, 0, :], Alu.subtract)
    nc.vector.tensor_tensor(acc2[:], acc2[:], psU2[:, 2, :], Alu.subtract)
    nc.vector.tensor_tensor(acc2[:], acc2[:], inv2[:], Alu.mult)
    nc.vector.tensor_tensor(acc2[:], acc2[:], lnt2[:], Alu.add)
    nc.vector.tensor_tensor(acc2[:], acc2[:], dm2[:, :], Alu.mult)
    nc.sync.dma_start(out=mi2_o[:, :], in_=acc2[:])

    # ---------------- pssm (overlaps mt1/tail PE work) ----------------
    cntp = small.tile([CNT_PER_CORE, NAA], f32)
    nc.vector.tensor_scalar(out=cntp[:], in0=cnt64[:], scalar1=pcntb[:, 0:1],
                            scalar2=None, op0=Alu.add)
    pssm_sb = small.tile([CNT_PER_CORE, NAA], f32)
    nc.scalar.activation(out=pssm_sb[:], in_=cntp[:], func=Act.Ln,
                         bias=eps_sb[0:CNT_PER_CORE, 0:1], scale=sc[:, 0:1])
    nc.sync.dma_start(out=pssm_o[:, :], in_=pssm_sb[:])

    # ---------------- conservation ----------------
    total = small.tile([CNT_PER_CORE, 1], f32)
    nc.vector.tensor_reduce(out=total[:], in_=cnt64[:], axis=mybir.AxisListType.X, op=Alu.add)
    tots = small.tile([CNT_PER_CORE, 1], f32)
    nc.vector.tensor_scalar(out=tots[:], in0=total[:], scalar1=1.0, scalar2=None, op0=Alu.max)
    invt = small.tile([CNT_PER_CORE, 1], f32)
    nc.vector.reciprocal(out=invt[:], in_=tots[:])
    ffreq = small.tile([CNT_PER_CORE, NAA], f32)
    nc.vector.tensor_scalar(out=ffreq[:], in0=cnt64[:], scalar1=invt[:, 0:1],
                            scalar2=None, op0=Alu.mult)
    lf = small.tile([CNT_PER_CORE, NAA], f32)
    nc.scalar.activation(out=lf[:], in_=ffreq[:], func=Act.Ln,
                         bias=eps_sb[0:CNT_PER_CORE, 0:1], scale=1.0)
    fl = small.tile([CNT_PER_CORE, NAA], f32)
    nc.vector.tensor_tensor(fl[:], ffreq[:], lf[:], Alu.mult)
    se = small.tile([CNT_PER_CORE, 1], f32)
    nc.vector.tensor_reduce(out=se[:], in_=fl[:], axis=mybir.AxisListType.X, op=Alu.add)
    consv = small.tile([CNT_PER_CORE, 1], f32)
    # cons = 1 + (sum f ln f)/ln(20)
    nc.vector.tensor_scalar(out=consv[:], in0=se[:], scalar1=1.0 / LN20, scalar2=1.0,
                            op0=Alu.mult, op1=Alu.add)
    mask = small.tile([CNT_PER_CORE, 1], f32)
    nc.vector.tensor_scalar(out=mask[:], in0=total[:], scalar1=0.0, scalar2=None, op0=Alu.is_gt)
    nc.vector.tensor_tensor(consv[:], consv[:], mask[:], Alu.mult)
    nc.sync.dma_start(out=cons_o[:, :], in_=consv[:])


_NC_CACHE = None


def _build_nc():
    global _NC_CACHE
    if _NC_CACHE is not None:
        return _NC_CACHE
    nc = bacc.Bacc("TRN2", target_bir_lowering=False)
    msa100 = nc.dram_tensor("msa100", [N_SEQS, MPOS], i32, kind="ExternalInput")
    msa_mi = nc.dram_tensor("msa_mi", [N_SEQS, POS_PER_CORE], i32, kind="ExternalInput")
    msa_mj = nc.dram_tensor("msa_mj", [N_SEQS, MJ_PER_CORE], i32, kind="ExternalInput")
    msa_cnt = nc.dram_tensor("msa_cnt", [N_SEQS, CNT_PER_CORE], i32, kind="ExternalInput")
    s_all = nc.dram_tensor("s_all", [P, 20], f32, kind="ExternalInput")
    dmask = nc.dram_tensor("dmask", [POS_PER_CORE, MPOS], f32, kind="ExternalInput")
    dmask2 = nc.dram_tensor("dmask2", [NTAIL, MJ_PER_CORE], f32, kind="ExternalInput")
    pc = nc.dram_tensor("pc", [1, 1], f32, kind="ExternalInput")
    pssm_o = nc.dram_tensor("pssm_part", [CNT_PER_CORE, NAA], f32, kind="ExternalOutput")
    cons_o = nc.dram_tensor("cons_part", [CNT_PER_CORE, 1], f32, kind="ExternalOutput")
    mi_o = nc.dram_tensor("mi_part", [POS_PER_CORE, MPOS], f32, kind="ExternalOutput")
    mi2_o = nc.dram_tensor("mi2_part", [NTAIL, MJ_PER_CORE], f32, kind="ExternalOutput")
    with tile.TileContext(nc) as tc:
        with ExitStack() as ctx:
            _emit_kernel(nc, tc, ctx,
                         (msa100, msa_mi, msa_mj, msa_cnt, s_all, dmask, dmask2, pc,
                          pssm_o, cons_o, mi_o, mi2_o))
    nc.compile()
    _NC_CACHE = nc
    return nc


def _host_inputs(msa, pc):
    msa = np.ascontiguousarray(np.asarray(msa), dtype=np.int32)
    pc_np = np.asarray(pc, dtype=np.float32).reshape(1, 1)
    # i-major S matrix: psum row r = m*21 + c
    s_arr = np.zeros((P, 20), np.float32)
    for m in range(6):                       # npos=6 tiles
        s_arr[NW * m: NW * m + NAA, m] = 1.0
        s_arr[NW * m + NAA, 6 + m] = 1.0
    for m in range(NTAIL):                   # npos=4 tail tile
        s_arr[NW * m: NW * m + NAA, 12 + m] = 1.0
        s_arr[NW * m + NAA, 16 + m] = 1.0
    msa100 = np.ascontiguousarray(msa[:, :MPOS])
    in_maps = []
    for c in range(NCORES):
        jcols = [(MJ_PER_CORE * c + t) if (MJ_PER_CORE * c + t) < MPOS else 0
                 for t in range(MJ_PER_CORE)]
        dmask = np.full((POS_PER_CORE, MPOS), 1.0 / LN2, np.float32)
        for t in range(POS_PER_CORE):
            dmask[t, POS_PER_CORE * c + t] = 0.0
        dmask2 = np.full((NTAIL, MJ_PER_CORE), 1.0 / LN2, np.float32)
        for u in range(NTAIL):
            for t in range(MJ_PER_CORE):
                if jcols[t] == 96 + u and MJ_PER_CORE * c + t < MPOS:
                    dmask2[u, t] = 0.0
        in_maps.append({
            "msa100": msa100,
            "msa_mi": np.ascontiguousarray(
                msa[:, POS_PER_CORE * c: POS_PER_CORE * (c + 1)]),
            "msa_mj": np.ascontiguousarray(msa[:, jcols]),
            "msa_cnt": np.ascontiguousarray(msa[:, CNT_PER_CORE * c: CNT_PER_CORE * (c + 1)]),
            "s_all": s_arr,
            "dmask": dmask,
            "dmask2": dmask2,
            "pc": pc_np,
        })
    return in_maps


def _run(msa, pc, **spmd_kwargs):
    nc = _build_nc()
    in_maps = _host_inputs(msa, pc)
    res = run_bass_kernel_spmd(nc, in_maps, core_ids=list(range(NCORES)), **spmd_kwargs)
    pssm = np.concatenate([res.results[c]["pssm_part"] for c in range(NCORES)], axis=0)
    cons = np.concatenate([res.results[c]["cons_part"][:, 0] for c in range(NCORES)], axis=0)
    rows = np.concatenate([res.results[c]["mi_part"] for c in range(NCORES)], axis=0)
    mi = np.zeros((SEQ_LEN, SEQ_LEN), np.float32)
    mi[:NCORES * POS_PER_CORE, :MPOS] = rows
    for c in range(NCORES):
        m2 = res.results[c]["mi2_part"]          # [4, 13]
        for t in range(MJ_PER_CORE):
            jc = MJ_PER_CORE * c + t
            if jc < MPOS:
                mi[96:MPOS, jc] = m2[:, t]
    return (pssm.astype(np.float32), cons.astype(np.float32), mi), res


def kernel(msa, pc):
    out, _ = _run(msa, pc)
    return out
